# revision 13
# baseline (speedup 1.0000x reference)
"""Trainium2 Bass kernel for nn_JointRelationModule (self-contained).

Math (per person p, softmax within one imgid group over the person dim):
    q = Wq x ; k = Wk x ; v = Wv x (+b*)        (1x1 conv over K=17 channels)
    S_p = q_p k_p^T / 64                        ([17,17] scores)
    attn = segment-softmax over persons (per imgid group, per (i,j) entry)
    out = relu(attn_p @ v_p + x_p)

Device formulation (per 7-person stack, BDK=119 channels on partitions):
  - G_s = Xt_s^T Xt_s accumulated over 32 hw-chunks of the host-uploaded
    transposed fp16 x (no on-device transposes of the big tensor).
  - masked block-diag G -> M1 = G @ BD(Wq^T/64) -> Z = Wkstack^T @ M1 gives
    scores^T for all 7 persons de-overlapped into a [17, 119] tile; exp on
    ACT writes E in fp16.
  - segment softmax without any gather/scatter DMAs: E^T via a tiny PE
    transpose, segment sums accumulate across stacks via kron(ind, I17)
    indicator matmuls (raggedness is data; program is SPMD-uniform),
    reciprocal on DVE, broadcast back via the transposed indicators.
  - V2 = RepWv^T @ A gives (attn_p Wv) replicated over block-rows; mask to
    block-diag and add I (folds the +x residual into the matmul); then
    out = (AT+I)^T @ x_fp16 per 512-col chunk, relu+bias on ACT/DVE
    alternating, fp16 output assembled per stack and written with one DMA.

Sharding: data-parallel over persons, split at imgid group boundaries
(8 cores), weights replicated. Output returned as f32 (host upcast).
"""

import math
import sys

import numpy as np

K = 17
HW = 4096  # 64*64
P_TOTAL = 512
N_CORES = 8
NORM = 64.0
BD = 7          # persons per stack
BDK = BD * K    # 119
D_CH = 128      # hw chunk for gram contraction
N_DCH = HW // D_CH  # 32
O_CH = 1024     # output chunk along hw dim

_cache: dict = {}


def _ensure_path():
    try:
        import concourse.bass  # noqa: F401
    except ImportError:
        for p in ("/opt/trn_rl_repo", "/root/.axon_site/_ro/trn_rl_repo"):
            if p not in sys.path:
                sys.path.insert(0, p)
        import concourse.bass  # noqa: F401


def _build(P_pad: int, n_gh: int):
    """Builds + compiles the per-core SPMD Bass program."""
    _ensure_path()
    import concourse.bacc as bacc
    import concourse.mybir as mybir
    import concourse.tile as tile

    f32 = mybir.dt.float32
    f16 = mybir.dt.float16
    Exp = mybir.ActivationFunctionType.Exp
    Relu = mybir.ActivationFunctionType.Relu
    Add = mybir.AluOpType.add
    Max = mybir.AluOpType.max

    S = P_pad // BD
    assert P_pad % BD == 0 and P_pad <= 128 and n_gh <= 2
    n_och = HW // O_CH  # 8

    nc = bacc.Bacc(
        "TRN2",
        target_bir_lowering=False,
        debug=False,
        enable_asserts=False,
        num_devices=N_CORES,
    )

    xt_d = nc.dram_tensor("xt", [128, S * N_DCH * BDK], f16, kind="ExternalInput")
    xp_d = nc.dram_tensor("xp", [BDK, S * HW], f16, kind="ExternalInput")
    wqbd_d = nc.dram_tensor("wqbd", [BDK, BDK], f32, kind="ExternalInput")
    wkst_d = nc.dram_tensor("wkst", [BDK, K], f32, kind="ExternalInput")
    wvrep_d = nc.dram_tensor("wvrep", [K, BDK], f16, kind="ExternalInput")
    mask_d = nc.dram_tensor("maskbd", [BDK, BDK], f32, kind="ExternalInput")
    id_d = nc.dram_tensor("idbd", [BDK, BDK], f32, kind="ExternalInput")
    id17_d = nc.dram_tensor("id17", [K, K], f16, kind="ExternalInput")
    corr_d = nc.dram_tensor("corrz", [K, S * BDK], f32, kind="ExternalInput")
    bv_d = nc.dram_tensor("bv17", [K, 1], f16, kind="ExternalInput")
    ind2_d = [nc.dram_tensor(f"ind2_{h}", [BDK, S * BDK], f16,
                             kind="ExternalInput") for h in range(n_gh)]
    ind2t_d = [nc.dram_tensor(f"ind2t_{h}", [BDK, S * BDK], f16,
                              kind="ExternalInput") for h in range(n_gh)]
    y_d = nc.dram_tensor("y", [BDK, S * HW], f16, kind="ExternalOutput")

    with tile.TileContext(nc) as tc:
        with (
            tc.tile_pool(name="xpool", bufs=1) as xpool,
            tc.tile_pool(name="cpool", bufs=1) as cpool,
            tc.tile_pool(name="wpool", bufs=2) as wpool,
            tc.tile_pool(name="fpool", bufs=1) as fpool,
            tc.tile_pool(name="opool", bufs=2) as opool,
            tc.tile_pool(name="pp", bufs=2, space="PSUM") as pp,
        ):
            # --- replicated constants (scalar/ACT queue) ---
            wqbd_t = cpool.tile([BDK, BDK], f32, name="wqbd_t", tag="wq")
            wkst_t = cpool.tile([BDK, K], f32, name="wkst_t", tag="wk")
            wvrep_t = cpool.tile([K, BDK], f16, name="wvrep_t", tag="wv")
            mask_t = cpool.tile([BDK, BDK], f32, name="mask_t", tag="mask")
            id_t = cpool.tile([BDK, BDK], f32, name="id_t", tag="id")
            id17_t = cpool.tile([K, K], f16, name="id17_t", tag="id17")
            corr_t = cpool.tile([K, S * BDK], f32, name="corr_t", tag="corr")
            bv_t = cpool.tile([K, 1], f16, name="bv_t", tag="bv")
            ind2_t = [cpool.tile([BDK, S * BDK], f16, name=f"ind2_{h}",
                                 tag=f"ind2_{h}") for h in range(n_gh)]
            ind2t_t = [cpool.tile([BDK, S * BDK], f16, name=f"ind2t_{h}",
                                  tag=f"ind2t_{h}") for h in range(n_gh)]
            # consts on scalar queue, chain-critical ones first
            nc.scalar.dma_start(mask_t[:], mask_d.ap())
            nc.scalar.dma_start(wqbd_t[:], wqbd_d.ap())
            nc.scalar.dma_start(wkst_t[:], wkst_d.ap())
            nc.scalar.dma_start(id17_t[:], id17_d.ap())
            nc.scalar.dma_start(corr_t[:], corr_d.ap())
            for h in range(n_gh):
                nc.scalar.dma_start(ind2_t[h][:], ind2_d[h].ap())
            nc.scalar.dma_start(wvrep_t[:], wvrep_d.ap())
            for h in range(n_gh):
                nc.scalar.dma_start(ind2t_t[h][:], ind2t_d[h].ap())
            nc.scalar.dma_start(id_t[:], id_d.ap())
            nc.scalar.dma_start(bv_t[:], bv_d.ap())

            # --- bulk loads: 2-stack chunks on sync+gpsimd only (never the
            # ACT queue - its triggers block compute). xt first: it gates
            # the gram; each chunk ~2MB (>=1MB for full DMA rate) and
            # completes individually so gram starts early.
            qs = [nc.sync, nc.gpsimd]
            sgroups = [(a, min(a + 2, S)) for a in range(0, S, 2)]
            xt_sb = xpool.tile([128, S * N_DCH * BDK], f16, name="xt_sb",
                               tag="xt", padded_shape=[128, S * HW])
            xpall = xpool.tile([BDK, S * HW], f16, name="xpall", tag="xp")
            for g, (sa, sb) in enumerate(sgroups):
                sl = slice(sa * N_DCH * BDK, sb * N_DCH * BDK)
                qs[g % 2].dma_start(xt_sb[:, sl], xt_d.ap()[:, sl])
            for g, (sa, sb) in enumerate(sgroups):
                sl = slice(sa * HW, sb * HW)
                qs[g % 2].dma_start(xpall[:, sl], xp_d.ap()[:, sl])

            eall = fpool.tile([K, S * BDK], f16, name="eall", tag="eall")
            a_t = fpool.tile([K, S * BDK], f16, name="a_t", tag="a")
            seg_ps = [pp.tile([BDK, K], f32, name=f"seg{h}", tag=f"seg{h}",
                              bufs=1) for h in range(n_gh)]

            # --- phase A: per stack gram -> chain -> exp -> E^T -> seg acc ---
            for s in range(S):
                zsl = slice(s * BDK, (s + 1) * BDK)
                g_ps = pp.tile([BDK, BDK], f32, name=f"g{s}", tag="g", bufs=1)
                base = s * N_DCH * BDK
                for c in range(N_DCH):
                    op = xt_sb[:, base + c * BDK: base + (c + 1) * BDK]
                    nc.tensor.matmul(
                        g_ps[:], op, op,
                        start=(c == 0), stop=(c == N_DCH - 1),
                    )
                g_sb = wpool.tile([BDK, BDK], f32, name="g_sb", tag="gsb")
                nc.vector.tensor_mul(g_sb[:], g_ps[:], mask_t[:])
                m1_ps = pp.tile([BDK, BDK], f32, name="m1", tag="tiny", bufs=1)
                nc.tensor.matmul(m1_ps[:], g_sb[:], wqbd_t[:], start=True, stop=True)
                m1_sb = wpool.tile([BDK, BDK], f32, name="m1_sb", tag="m1")
                nc.scalar.copy(m1_sb[:], m1_ps[:])
                z_ps = pp.tile([K, BDK], f32, name="z", tag="tiny", bufs=1)
                nc.tensor.matmul(z_ps[:], wkst_t[:], m1_sb[:], start=True, stop=True)
                zc_sb = wpool.tile([K, BDK], f32, name="zc_sb", tag="zc")
                nc.vector.tensor_add(zc_sb[:], z_ps[:], corr_t[:, zsl])
                nc.scalar.activation(eall[:, zsl], zc_sb[:], Exp)
                et_ps = pp.tile([BDK, K], f16, name="et", tag="ops", bufs=2)
                nc.tensor.transpose(et_ps[:], eall[:, zsl], id17_t[:])
                et_sb = wpool.tile([BDK, K], f16, name="et_sb", tag="et")
                nc.vector.tensor_copy(et_sb[:], et_ps[:])
                for h in range(n_gh):
                    nc.tensor.matmul(
                        seg_ps[h][:], ind2_t[h][:, zsl], et_sb[:],
                        start=(s == 0), stop=(s == S - 1),
                    )

            # --- phase C: reciprocal of group sums; broadcast back per stack ---
            # clamp the reciprocal so empty group slots (seg=0) stay finite in
            # fp16; their zero indicator columns make them exact zeros later.
            inv_sb = []
            for h in range(n_gh):
                segc = fpool.tile([BDK, K], f32, name=f"segc{h}", tag=f"segc{h}")
                nc.vector.tensor_scalar_max(segc[:], seg_ps[h][:], 1e-30)
                invf = fpool.tile([BDK, K], f32, name=f"invf{h}", tag=f"invf{h}")
                nc.vector.reciprocal(invf[:], segc[:])
                inv = fpool.tile([BDK, K], f16, name=f"inv{h}", tag=f"inv{h}")
                nc.vector.tensor_scalar_min(inv[:], invf[:], 60000.0)
                inv_sb.append(inv)

            # --- phase D per stack: invB -> A -> AT(+I) -> out chunks ---
            yall = xpool.tile([BDK, S * HW], f16, name="yall", tag="xt")
            for s in range(S):
                zsl = slice(s * BDK, (s + 1) * BDK)
                invb_ps = pp.tile([K, BDK], f32, name="invb", tag="tiny", bufs=1)
                for h in range(n_gh):
                    nc.tensor.matmul(
                        invb_ps[:], inv_sb[h][:], ind2t_t[h][:, zsl],
                        start=(h == 0), stop=(h == n_gh - 1),
                    )
                nc.vector.tensor_mul(a_t[:, zsl], eall[:, zsl], invb_ps[:])

                v2_ps = pp.tile([BDK, BDK], f32, name="v2", tag="tiny", bufs=1)
                nc.tensor.matmul(v2_ps[:], wvrep_t[:], a_t[:, zsl],
                                 start=True, stop=True)
                t1_sb = wpool.tile([BDK, BDK], f32, name="t1_sb", tag="t1")
                nc.vector.tensor_mul(t1_sb[:], v2_ps[:], mask_t[:])
                at_sb = wpool.tile([BDK, BDK], f16, name="at_sb", tag="atsb")
                nc.vector.tensor_add(at_sb[:], t1_sb[:], id_t[:])
                av_ps = pp.tile([BDK, 1], f32, name="av", tag="tiny", bufs=1)
                nc.tensor.matmul(av_ps[:], a_t[:, zsl], bv_t[:],
                                 start=True, stop=True)
                av_sb = wpool.tile([BDK, 1], f32, name="av_sb", tag="avsb")
                nc.vector.tensor_copy(av_sb[:], av_ps[:])

                for oc in range(n_och):
                    osl = slice(s * HW + O_CH * oc, s * HW + O_CH * (oc + 1))
                    o_ps = pp.tile([BDK, O_CH], f32, name="o_ps", tag="ops",
                                   bufs=2)
                    # two matmuls per chunk: a PSUM write can't cross a bank
                    for hh in range(2):
                        hsl = slice(s * HW + O_CH * oc + 512 * hh,
                                    s * HW + O_CH * oc + 512 * (hh + 1))
                        nc.tensor.matmul(
                            o_ps[:, 512 * hh:512 * (hh + 1)], at_sb[:],
                            xpall[:, hsl], start=True, stop=True,
                        )
                    if oc % 2 == 0:
                        nc.scalar.activation(yall[:, osl], o_ps[:], Relu,
                                             bias=av_sb[:, 0:1])
                    else:
                        nc.vector.tensor_scalar(
                            yall[:, osl], o_ps[:], av_sb[:, 0:1], 0.0,
                            op0=Add, op1=Max,
                        )
                for g, (sa, sb) in enumerate(sgroups):
                    if s == sb - 1:
                        sl = slice(sa * HW, sb * HW)
                        qs[g % 2].dma_start(y_d.ap()[:, sl], yall[:, sl])

    nc.compile()
    return nc


def _get_compiled(P_pad: int, n_gh: int):
    key = (P_pad, n_gh)
    if key not in _cache:
        _cache[key] = _build(P_pad, n_gh)
    return _cache[key]


def _plan(ids: np.ndarray):
    """Split persons into N_CORES contiguous chunks at imgid boundaries."""
    change = np.flatnonzero(np.diff(ids)) + 1
    allb = np.concatenate([[0], change, [P_TOTAL]]).astype(np.int64)
    bounds = [0]
    for ci in range(1, N_CORES):
        target = P_TOTAL * ci / N_CORES
        cand = allb[allb > bounds[-1]]
        if len(cand) == 0:
            bounds.append(bounds[-1])
        else:
            bounds.append(int(cand[np.argmin(np.abs(cand - target))]))
    bounds.append(P_TOTAL)
    sizes = np.diff(bounds)
    P_max = int(sizes.max())
    P_pad = max(BD, BD * math.ceil(P_max / BD))
    g_max = 0
    for ci in range(N_CORES):
        a, b = bounds[ci], bounds[ci + 1]
        g_max = max(g_max, len(np.unique(ids[a:b])))
    n_gh = math.ceil((g_max + 1) / BD)
    return bounds, P_pad, n_gh


def _prepare(inputs: dict):
    x = np.asarray(inputs["kpt_feat"], dtype=np.float32).reshape(P_TOTAL, K, HW)
    ids = np.asarray(inputs["imgid"]).astype(np.int64)
    Wq = np.asarray(inputs["Wq"], np.float32)
    Wk = np.asarray(inputs["Wk"], np.float32)
    Wv = np.asarray(inputs["Wv"], np.float32)
    bq = np.asarray(inputs["bq"], np.float32)
    bk = np.asarray(inputs["bk"], np.float32)
    bv = np.asarray(inputs["bv"], np.float32)

    bounds, P_pad, n_gh = _plan(ids)
    S = P_pad // BD

    def bd(m):
        out = np.zeros((BDK, BDK), dtype=np.float32)
        for j in range(BD):
            out[K * j:K * (j + 1), K * j:K * (j + 1)] = m
        return out

    wqbd = bd((Wq.T / NORM).astype(np.float32))
    wkst = np.tile(Wk.T.astype(np.float32), (BD, 1))          # [119, 17]
    wvrep = np.tile(Wv.astype(np.float16), (1, BD))           # [17, 119]
    maskbd = bd(np.ones((K, K), np.float32))
    idbd = np.eye(BDK, dtype=np.float32)
    id17 = np.eye(K, dtype=np.float16)
    bv17 = bv.reshape(K, 1).astype(np.float16)
    i17f = np.eye(K, dtype=np.float32)

    have_bias = bool(np.any(bq) or np.any(bk))
    if have_bias:
        xsum = x.sum(axis=2)                    # [P, K]
        qx = xsum @ Wq.T                        # [P, i]
        kx = xsum @ Wk.T                        # [P, m]
        corr_all = (
            bk[None, :, None] * qx[:, None, :]
            + bq[None, None, :] * kx[:, :, None]
            + HW * (bq[None, None, :] * bk[None, :, None])
        ) / NORM                                # [P, m, i]
        corr_all = corr_all.astype(np.float32)
    else:
        corr_all = np.zeros((P_TOTAL, K, K), dtype=np.float32)
    # [m, i] layout to match corr_all; C[i,a] indexed here as [a==m, i]? no:
    # corr_all is [P, m, i] with value for ST[m, i] = scores[i, m]; the shift
    # for scores[i, m] is hw*(Wq_i . Wk_m)/64 -> [m, i] = (Wk Wq^T * hw/64)
    cshift = (HW / NORM) * (Wk @ Wq.T)          # [m, i]

    in_maps = []
    for ci in range(N_CORES):
        a, b = bounds[ci], bounds[ci + 1]
        pc = b - a
        xpad = np.zeros((P_pad, K, HW), dtype=np.float32)
        if pc:
            xpad[:pc] = x[a:b]
        x16 = xpad.astype(np.float16)
        # xt: [128(hw within chunk), S, 32(chunk), 119] from [S,119,32,128]
        xt = np.ascontiguousarray(
            x16.reshape(S, BDK, N_DCH, D_CH).transpose(3, 0, 2, 1)
        ).reshape(128, S * N_DCH * BDK)
        # xp: all stacks side by side on the same 119 partitions -> 80KB rows
        xp = np.ascontiguousarray(
            x16.reshape(S, BDK, HW).transpose(1, 0, 2)
        ).reshape(BDK, S * HW)

        # corr bias in Z layout, minus the data-independent expected-score
        # shift C[i,a] = hw*(Wq_i . Wk_a)/64 (cancels exactly in the segment
        # softmax, keeps exp() in fp16 range). Pads (x=0) get plain zero.
        czp = np.zeros((P_pad, K, K), np.float32)
        if pc:
            czp[:pc] = corr_all[a:b] - cshift[None, :, :]
        corrz = np.ascontiguousarray(
            czp.transpose(1, 0, 2)).reshape(K, S * BDK)

        # local group index per person; pads -> dummy group g_max_local
        lg = np.full(P_pad, 0, np.int64)
        ng_local = 0
        if pc:
            _, lgc = np.unique(ids[a:b], return_inverse=True)
            lg[:pc] = lgc
            ng_local = int(lgc.max()) + 1
        lg[pc:] = ng_local  # dummy group for padding
        ind_full = np.zeros((P_pad, BD * n_gh), np.float32)
        ind_full[np.arange(P_pad), lg] = 1.0
        ind2, ind2t = [], []
        for h in range(n_gh):
            ind_h = ind_full[:, BD * h:BD * (h + 1)]          # [P_pad, 7]
            arr = np.einsum('sjg,ik->sjigk',
                            ind_h.reshape(S, BD, BD),
                            i17f).reshape(S, BDK, BDK)
            ind2.append(np.ascontiguousarray(
                arr.transpose(1, 0, 2)).reshape(BDK, S * BDK).astype(np.float16))
            ind2t.append(np.ascontiguousarray(
                arr.transpose(2, 0, 1)).reshape(BDK, S * BDK).astype(np.float16))

        im = {
            "xt": xt,
            "xp": xp,
            "wqbd": wqbd,
            "wkst": wkst,
            "wvrep": wvrep,
            "maskbd": maskbd,
            "idbd": idbd,
            "id17": id17,
            "corrz": corrz,
            "bv17": bv17,
        }
        for h in range(n_gh):
            im[f"ind2_{h}"] = ind2[h]
            im[f"ind2t_{h}"] = ind2t[h]
        in_maps.append(im)
    return in_maps, bounds, P_pad, n_gh


def _gather(results, bounds, P_pad):
    S = P_pad // BD
    out = np.empty((P_TOTAL, K, 64, 64), dtype=np.float32)
    for ci in range(N_CORES):
        a, b = bounds[ci], bounds[ci + 1]
        pc = b - a
        if pc:
            y = results[ci]["y"].astype(np.float32)           # [119, S*HW]
            y = y.reshape(BDK, S, HW).transpose(1, 0, 2)      # [S, 119, HW]
            out[a:b] = y.reshape(P_pad, K, 64, 64)[:pc]
    return out


def _run(inputs: dict, trace: bool = False):
    _ensure_path()
    from concourse.bass_utils import run_bass_kernel_spmd

    in_maps, bounds, P_pad, n_gh = _prepare(inputs)
    nc = _get_compiled(P_pad, n_gh)
    res = run_bass_kernel_spmd(nc, in_maps, list(range(N_CORES)), trace=trace)
    return _gather(res.results, bounds, P_pad), res


def kernel(**inputs) -> np.ndarray:
    out, _ = _run(inputs, trace=False)
    return out


# revision 15
# speedup vs baseline: 1.0683x; 1.0683x over previous
"""Trainium2 Bass kernel for nn_JointRelationModule (self-contained).

Math (per person p, softmax within one imgid group over the person dim):
    q = Wq x ; k = Wk x ; v = Wv x (+b*)        (1x1 conv over K=17 channels)
    S_p = q_p k_p^T / 64                        ([17,17] scores)
    attn = segment-softmax over persons (per imgid group, per (i,j) entry)
    out = relu(attn_p @ v_p + x_p)

Device formulation (per 7-person stack, BDK=119 channels on partitions):
  - G_s = Xt_s^T Xt_s accumulated over 32 hw-chunks of the host-uploaded
    transposed fp16 x (no on-device transposes of the big tensor).
  - masked block-diag G -> M1 = G @ BD(Wq^T/64) -> Z = Wkstack^T @ M1 gives
    scores^T for all 7 persons de-overlapped into a [17, 119] tile; exp on
    ACT writes E in fp16.
  - segment softmax without any gather/scatter DMAs: E^T via a tiny PE
    transpose, segment sums accumulate across stacks via kron(ind, I17)
    indicator matmuls (raggedness is data; program is SPMD-uniform),
    reciprocal on DVE, broadcast back via the transposed indicators.
  - V2 = RepWv^T @ A gives (attn_p Wv) replicated over block-rows; mask to
    block-diag and add I (folds the +x residual into the matmul); then
    out = (AT+I)^T @ x_fp16 per 512-col chunk, relu+bias on ACT/DVE
    alternating, fp16 output assembled per stack and written with one DMA.

Sharding: data-parallel over persons, split at imgid group boundaries
(8 cores), weights replicated. Output returned as f32 (host upcast).
"""

import math
import sys

import numpy as np

K = 17
HW = 4096  # 64*64
P_TOTAL = 512
N_CORES = 8
NORM = 64.0
BD = 7          # persons per stack
BDK = BD * K    # 119
D_CH = 128      # hw chunk for gram contraction
N_DCH = HW // D_CH  # 32
O_CH = 1024     # output chunk along hw dim

_cache: dict = {}


def _ensure_path():
    try:
        import concourse.bass  # noqa: F401
    except ImportError:
        for p in ("/opt/trn_rl_repo", "/root/.axon_site/_ro/trn_rl_repo"):
            if p not in sys.path:
                sys.path.insert(0, p)
        import concourse.bass  # noqa: F401


def _build(P_pad: int, n_gh: int):
    """Builds + compiles the per-core SPMD Bass program."""
    _ensure_path()
    import concourse.bacc as bacc
    import concourse.mybir as mybir
    import concourse.tile as tile

    f32 = mybir.dt.float32
    f16 = mybir.dt.float16
    Exp = mybir.ActivationFunctionType.Exp
    Relu = mybir.ActivationFunctionType.Relu
    Add = mybir.AluOpType.add
    Max = mybir.AluOpType.max

    S = P_pad // BD
    assert P_pad % BD == 0 and P_pad <= 128 and n_gh <= 2
    n_och = HW // O_CH  # 8

    nc = bacc.Bacc(
        "TRN2",
        target_bir_lowering=False,
        debug=False,
        enable_asserts=False,
        num_devices=N_CORES,
    )

    xp_d = nc.dram_tensor("xp", [128, S * HW], f16, kind="ExternalInput")
    wqbd_d = nc.dram_tensor("wqbd", [BDK, BDK], f32, kind="ExternalInput")
    wkst_d = nc.dram_tensor("wkst", [BDK, K], f32, kind="ExternalInput")
    wvrep_d = nc.dram_tensor("wvrep", [K, BDK], f16, kind="ExternalInput")
    mask_d = nc.dram_tensor("maskbd", [BDK, BDK], f32, kind="ExternalInput")
    id_d = nc.dram_tensor("idbd", [BDK, BDK], f32, kind="ExternalInput")
    id17_d = nc.dram_tensor("id17", [K, K], f16, kind="ExternalInput")
    corr_d = nc.dram_tensor("corrz", [K, S * BDK], f32, kind="ExternalInput")
    bv_d = nc.dram_tensor("bv17", [K, 1], f16, kind="ExternalInput")
    ind2_d = [nc.dram_tensor(f"ind2_{h}", [BDK, S * BDK], f16,
                             kind="ExternalInput") for h in range(n_gh)]
    ind2t_d = [nc.dram_tensor(f"ind2t_{h}", [BDK, S * BDK], f16,
                              kind="ExternalInput") for h in range(n_gh)]
    y_d = nc.dram_tensor("y", [BDK, S * HW], f16, kind="ExternalOutput")

    with tile.TileContext(nc) as tc:
        with (
            tc.tile_pool(name="xpool", bufs=1) as xpool,
            tc.tile_pool(name="cpool", bufs=1) as cpool,
            tc.tile_pool(name="wpool", bufs=2) as wpool,
            tc.tile_pool(name="fpool", bufs=1) as fpool,
            tc.tile_pool(name="opool", bufs=2) as opool,
            tc.tile_pool(name="pp", bufs=2, space="PSUM") as pp,
        ):
            # --- replicated constants (scalar/ACT queue) ---
            wqbd_t = cpool.tile([BDK, BDK], f32, name="wqbd_t", tag="wq")
            wkst_t = cpool.tile([BDK, K], f32, name="wkst_t", tag="wk")
            wvrep_t = cpool.tile([K, BDK], f16, name="wvrep_t", tag="wv")
            mask_t = cpool.tile([BDK, BDK], f32, name="mask_t", tag="mask")
            id_t = cpool.tile([BDK, BDK], f32, name="id_t", tag="id")
            id17_t = cpool.tile([K, K], f16, name="id17_t", tag="id17")
            corr_t = cpool.tile([K, S * BDK], f32, name="corr_t", tag="corr")
            bv_t = cpool.tile([K, 1], f16, name="bv_t", tag="bv")
            ind2_t = [cpool.tile([BDK, S * BDK], f16, name=f"ind2_{h}",
                                 tag=f"ind2_{h}") for h in range(n_gh)]
            ind2t_t = [cpool.tile([BDK, S * BDK], f16, name=f"ind2t_{h}",
                                  tag=f"ind2t_{h}") for h in range(n_gh)]
            # tiny consts on scalar; bigger softmax-critical consts go on
            # gpsimd BEFORE the bulk loads so they can't get stuck behind them
            nc.scalar.dma_start(mask_t[:], mask_d.ap())
            nc.scalar.dma_start(wqbd_t[:], wqbd_d.ap())
            nc.scalar.dma_start(wkst_t[:], wkst_d.ap())
            nc.scalar.dma_start(id17_t[:], id17_d.ap())
            nc.scalar.dma_start(wvrep_t[:], wvrep_d.ap())
            nc.scalar.dma_start(id_t[:], id_d.ap())
            nc.scalar.dma_start(bv_t[:], bv_d.ap())
            nc.gpsimd.dma_start(corr_t[:], corr_d.ap())
            for h in range(n_gh):
                nc.gpsimd.dma_start(ind2_t[h][:], ind2_d[h].ap())
            for h in range(n_gh):
                nc.gpsimd.dma_start(ind2t_t[h][:], ind2t_d[h].ap())

            # --- bulk load: xp only (x is uploaded ONCE); per-stack XBAR
            # dma-transposes (SBUF->SBUF, no HBM) build the hw-major copy
            # for the gram: xtb[p, 128c+? ...] = out[p, c, r] = xp[r, 128c+p]
            qs = [nc.sync, nc.gpsimd]
            sgroups = [(a, min(a + 2, S)) for a in range(0, S, 2)]
            xpall = xpool.tile([128, S * HW], f16, name="xpall", tag="xp")
            xtb = [xpool.tile([128, HW], f16, name=f"xtb{s}", tag=f"xtb{s}")
                   for s in range(S)]
            for g, (sa, sb) in enumerate(sgroups):
                sl = slice(sa * HW, sb * HW)
                qs[g % 2].dma_start(xpall[:, sl], xp_d.ap()[:, sl])
            for s in range(S):
                nc.sync.dma_start(
                    xtb[s].rearrange("p (c r) -> p c r", c=N_DCH),
                    xpall[:, s * HW:(s + 1) * HW], transpose=True,
                )

            eall = fpool.tile([K, S * BDK], f16, name="eall", tag="eall")
            a_t = fpool.tile([K, S * BDK], f16, name="a_t", tag="a")
            seg_ps = [pp.tile([BDK, K], f32, name=f"seg{h}", tag=f"seg{h}",
                              bufs=1) for h in range(n_gh)]

            # --- phase A: per stack gram -> chain -> exp -> E^T -> seg acc ---
            for s in range(S):
                zsl = slice(s * BDK, (s + 1) * BDK)
                g_ps = pp.tile([BDK, BDK], f32, name=f"g{s}", tag="g", bufs=1)
                for c in range(N_DCH):
                    op = xtb[s][:, c * D_CH: c * D_CH + BDK]
                    nc.tensor.matmul(
                        g_ps[:], op, op,
                        start=(c == 0), stop=(c == N_DCH - 1),
                    )
                g_sb = wpool.tile([BDK, BDK], f32, name="g_sb", tag="gsb")
                nc.vector.tensor_mul(g_sb[:], g_ps[:], mask_t[:])
                m1_ps = pp.tile([BDK, BDK], f32, name="m1", tag="tiny", bufs=1)
                nc.tensor.matmul(m1_ps[:], g_sb[:], wqbd_t[:], start=True, stop=True)
                m1_sb = wpool.tile([BDK, BDK], f32, name="m1_sb", tag="m1")
                nc.scalar.copy(m1_sb[:], m1_ps[:])
                z_ps = pp.tile([K, BDK], f32, name="z", tag="tiny", bufs=1)
                nc.tensor.matmul(z_ps[:], wkst_t[:], m1_sb[:], start=True, stop=True)
                zc_sb = wpool.tile([K, BDK], f32, name="zc_sb", tag="zc")
                nc.vector.tensor_add(zc_sb[:], z_ps[:], corr_t[:, zsl])
                nc.scalar.activation(eall[:, zsl], zc_sb[:], Exp)
                et_ps = pp.tile([BDK, K], f16, name="et", tag="ops", bufs=2)
                nc.tensor.transpose(et_ps[:], eall[:, zsl], id17_t[:])
                et_sb = wpool.tile([BDK, K], f16, name="et_sb", tag="et")
                nc.vector.tensor_copy(et_sb[:], et_ps[:])
                for h in range(n_gh):
                    nc.tensor.matmul(
                        seg_ps[h][:], ind2_t[h][:, zsl], et_sb[:],
                        start=(s == 0), stop=(s == S - 1),
                    )

            # --- phase C: reciprocal of group sums; broadcast back per stack ---
            # clamp the reciprocal so empty group slots (seg=0) stay finite in
            # fp16; their zero indicator columns make them exact zeros later.
            inv_sb = []
            for h in range(n_gh):
                segc = fpool.tile([BDK, K], f32, name=f"segc{h}", tag=f"segc{h}")
                nc.vector.tensor_scalar_max(segc[:], seg_ps[h][:], 1e-30)
                invf = fpool.tile([BDK, K], f32, name=f"invf{h}", tag=f"invf{h}")
                nc.vector.reciprocal(invf[:], segc[:])
                inv = fpool.tile([BDK, K], f16, name=f"inv{h}", tag=f"inv{h}")
                nc.vector.tensor_scalar_min(inv[:], invf[:], 60000.0)
                inv_sb.append(inv)

            # --- phase D per stack: invB -> A -> AT(+I) -> out chunks ---
            for s in range(S):
                zsl = slice(s * BDK, (s + 1) * BDK)
                invb_ps = pp.tile([K, BDK], f32, name="invb", tag="tiny", bufs=1)
                for h in range(n_gh):
                    nc.tensor.matmul(
                        invb_ps[:], inv_sb[h][:], ind2t_t[h][:, zsl],
                        start=(h == 0), stop=(h == n_gh - 1),
                    )
                nc.vector.tensor_mul(a_t[:, zsl], eall[:, zsl], invb_ps[:])

                v2_ps = pp.tile([BDK, BDK], f32, name="v2", tag="tiny", bufs=1)
                nc.tensor.matmul(v2_ps[:], wvrep_t[:], a_t[:, zsl],
                                 start=True, stop=True)
                t1_sb = wpool.tile([BDK, BDK], f32, name="t1_sb", tag="t1")
                nc.vector.tensor_mul(t1_sb[:], v2_ps[:], mask_t[:])
                at_sb = wpool.tile([BDK, BDK], f16, name="at_sb", tag="atsb")
                nc.vector.tensor_add(at_sb[:], t1_sb[:], id_t[:])
                av_ps = pp.tile([BDK, 1], f32, name="av", tag="tiny", bufs=1)
                nc.tensor.matmul(av_ps[:], a_t[:, zsl], bv_t[:],
                                 start=True, stop=True)
                av_sb = wpool.tile([BDK, 1], f32, name="av_sb", tag="avsb")
                nc.vector.tensor_copy(av_sb[:], av_ps[:])

                y_sb = xpool.tile([BDK, HW], f16, name=f"y{s}", tag=f"xtb{s}")
                for oc in range(n_och):
                    osl = slice(O_CH * oc, O_CH * (oc + 1))
                    o_ps = pp.tile([BDK, O_CH], f32, name="o_ps", tag="ops",
                                   bufs=2)
                    # two matmuls per chunk: a PSUM write can't cross a bank
                    for hh in range(2):
                        hsl = slice(s * HW + O_CH * oc + 512 * hh,
                                    s * HW + O_CH * oc + 512 * (hh + 1))
                        nc.tensor.matmul(
                            o_ps[:, 512 * hh:512 * (hh + 1)], at_sb[:],
                            xpall[0:BDK, hsl], start=True, stop=True,
                        )
                    if oc % 2 == 0:
                        nc.scalar.activation(y_sb[:, osl], o_ps[:], Relu,
                                             bias=av_sb[:, 0:1])
                    else:
                        nc.vector.tensor_scalar(
                            y_sb[:, osl], o_ps[:], av_sb[:, 0:1], 0.0,
                            op0=Add, op1=Max,
                        )
                qs[s % 2].dma_start(
                    y_d.ap()[:, s * HW:(s + 1) * HW], y_sb[:])

    nc.compile()
    return nc


def _get_compiled(P_pad: int, n_gh: int):
    key = (P_pad, n_gh)
    if key not in _cache:
        _cache[key] = _build(P_pad, n_gh)
    return _cache[key]


def _plan(ids: np.ndarray):
    """Split persons into N_CORES contiguous chunks at imgid boundaries."""
    change = np.flatnonzero(np.diff(ids)) + 1
    allb = np.concatenate([[0], change, [P_TOTAL]]).astype(np.int64)
    bounds = [0]
    for ci in range(1, N_CORES):
        target = P_TOTAL * ci / N_CORES
        cand = allb[allb > bounds[-1]]
        if len(cand) == 0:
            bounds.append(bounds[-1])
        else:
            bounds.append(int(cand[np.argmin(np.abs(cand - target))]))
    bounds.append(P_TOTAL)
    sizes = np.diff(bounds)
    P_max = int(sizes.max())
    P_pad = max(BD, BD * math.ceil(P_max / BD))
    g_max = 0
    for ci in range(N_CORES):
        a, b = bounds[ci], bounds[ci + 1]
        g_max = max(g_max, len(np.unique(ids[a:b])))
    n_gh = math.ceil((g_max + 1) / BD)
    return bounds, P_pad, n_gh


def _prepare(inputs: dict):
    x = np.asarray(inputs["kpt_feat"], dtype=np.float32).reshape(P_TOTAL, K, HW)
    ids = np.asarray(inputs["imgid"]).astype(np.int64)
    Wq = np.asarray(inputs["Wq"], np.float32)
    Wk = np.asarray(inputs["Wk"], np.float32)
    Wv = np.asarray(inputs["Wv"], np.float32)
    bq = np.asarray(inputs["bq"], np.float32)
    bk = np.asarray(inputs["bk"], np.float32)
    bv = np.asarray(inputs["bv"], np.float32)

    bounds, P_pad, n_gh = _plan(ids)
    S = P_pad // BD

    def bd(m):
        out = np.zeros((BDK, BDK), dtype=np.float32)
        for j in range(BD):
            out[K * j:K * (j + 1), K * j:K * (j + 1)] = m
        return out

    wqbd = bd((Wq.T / NORM).astype(np.float32))
    wkst = np.tile(Wk.T.astype(np.float32), (BD, 1))          # [119, 17]
    wvrep = np.tile(Wv.astype(np.float16), (1, BD))           # [17, 119]
    maskbd = bd(np.ones((K, K), np.float32))
    idbd = np.eye(BDK, dtype=np.float32)
    id17 = np.eye(K, dtype=np.float16)
    bv17 = bv.reshape(K, 1).astype(np.float16)
    i17f = np.eye(K, dtype=np.float32)

    have_bias = bool(np.any(bq) or np.any(bk))
    if have_bias:
        xsum = x.sum(axis=2)                    # [P, K]
        qx = xsum @ Wq.T                        # [P, i]
        kx = xsum @ Wk.T                        # [P, m]
        corr_all = (
            bk[None, :, None] * qx[:, None, :]
            + bq[None, None, :] * kx[:, :, None]
            + HW * (bq[None, None, :] * bk[None, :, None])
        ) / NORM                                # [P, m, i]
        corr_all = corr_all.astype(np.float32)
    else:
        corr_all = np.zeros((P_TOTAL, K, K), dtype=np.float32)
    # [m, i] layout to match corr_all; C[i,a] indexed here as [a==m, i]? no:
    # corr_all is [P, m, i] with value for ST[m, i] = scores[i, m]; the shift
    # for scores[i, m] is hw*(Wq_i . Wk_m)/64 -> [m, i] = (Wk Wq^T * hw/64)
    cshift = (HW / NORM) * (Wk @ Wq.T)          # [m, i]

    in_maps = []
    for ci in range(N_CORES):
        a, b = bounds[ci], bounds[ci + 1]
        pc = b - a
        xpad = np.zeros((P_pad, K, HW), dtype=np.float32)
        if pc:
            xpad[:pc] = x[a:b]
        x16 = xpad.astype(np.float16)
        # xp: all stacks side by side on the same partitions -> 80KB rows;
        # padded to 128 rows so the on-device XBAR transpose reads zeros
        xp = np.zeros((128, S * HW), np.float16)
        xp[:BDK] = np.ascontiguousarray(
            x16.reshape(S, BDK, HW).transpose(1, 0, 2)
        ).reshape(BDK, S * HW)

        # corr bias in Z layout, minus the data-independent expected-score
        # shift C[i,a] = hw*(Wq_i . Wk_a)/64 (cancels exactly in the segment
        # softmax, keeps exp() in fp16 range). Pads (x=0) get plain zero.
        czp = np.zeros((P_pad, K, K), np.float32)
        if pc:
            czp[:pc] = corr_all[a:b] - cshift[None, :, :]
        corrz = np.ascontiguousarray(
            czp.transpose(1, 0, 2)).reshape(K, S * BDK)

        # local group index per person; pads -> dummy group g_max_local
        lg = np.full(P_pad, 0, np.int64)
        ng_local = 0
        if pc:
            _, lgc = np.unique(ids[a:b], return_inverse=True)
            lg[:pc] = lgc
            ng_local = int(lgc.max()) + 1
        lg[pc:] = ng_local  # dummy group for padding
        ind_full = np.zeros((P_pad, BD * n_gh), np.float32)
        ind_full[np.arange(P_pad), lg] = 1.0
        ind2, ind2t = [], []
        for h in range(n_gh):
            ind_h = ind_full[:, BD * h:BD * (h + 1)]          # [P_pad, 7]
            arr = np.einsum('sjg,ik->sjigk',
                            ind_h.reshape(S, BD, BD),
                            i17f).reshape(S, BDK, BDK)
            ind2.append(np.ascontiguousarray(
                arr.transpose(1, 0, 2)).reshape(BDK, S * BDK).astype(np.float16))
            ind2t.append(np.ascontiguousarray(
                arr.transpose(2, 0, 1)).reshape(BDK, S * BDK).astype(np.float16))

        im = {
            "xp": xp,
            "wqbd": wqbd,
            "wkst": wkst,
            "wvrep": wvrep,
            "maskbd": maskbd,
            "idbd": idbd,
            "id17": id17,
            "corrz": corrz,
            "bv17": bv17,
        }
        for h in range(n_gh):
            im[f"ind2_{h}"] = ind2[h]
            im[f"ind2t_{h}"] = ind2t[h]
        in_maps.append(im)
    return in_maps, bounds, P_pad, n_gh


def _gather(results, bounds, P_pad):
    S = P_pad // BD
    out = np.empty((P_TOTAL, K, 64, 64), dtype=np.float32)
    for ci in range(N_CORES):
        a, b = bounds[ci], bounds[ci + 1]
        pc = b - a
        if pc:
            y = results[ci]["y"].astype(np.float32)           # [119, S*HW]
            y = y.reshape(BDK, S, HW).transpose(1, 0, 2)      # [S, 119, HW]
            out[a:b] = y.reshape(P_pad, K, 64, 64)[:pc]
    return out


def _run(inputs: dict, trace: bool = False):
    _ensure_path()
    from concourse.bass_utils import run_bass_kernel_spmd

    in_maps, bounds, P_pad, n_gh = _prepare(inputs)
    nc = _get_compiled(P_pad, n_gh)
    res = run_bass_kernel_spmd(nc, in_maps, list(range(N_CORES)), trace=trace)
    return _gather(res.results, bounds, P_pad), res


def kernel(**inputs) -> np.ndarray:
    out, _ = _run(inputs, trace=False)
    return out
